# revision 29
# baseline (speedup 1.0000x reference)
"""Trainium2 Bass kernel for nn_NeuralCellularAutomata2 (B16,H64,W64,C256).

Self-contained: hardcodes shapes/sharding. Strategy:
 - data-parallel over batch: 16 images -> 8 cores x 2 images
 - the dominant costs through the axon PJRT path are (a) host<->device
   transfer bytes and (b) a per-BIR-instruction dispatch overhead
   (~50 us/instruction: each call re-lowers and re-loads the program), so
   the kernel both compresses the wire format AND minimizes program size
   with hardware For_i loops (images, row-tiles, chunks), staging matmul
   stationary operands into fixed tiles where register offsets are not
   allowed:
     h ships as fp8 e3m4 NHWC pixel-major chunks (one cast + byte shuffle
     on host); the device transposes to channel-major via PE identity
     matmuls and zero-pads in SBUF.
     the depthwise 3x3 conv runs as 9 shifted DVE multiply-accumulates
     (per-partition tap scalars), feeding a K=768 1x1 matmul; W1 ships
     fp8 e4m3 x16, descaled by the GELU's input scale.
     qkv folds host-side into A = Wq^T Wk / sqrt(C) (x256 in fp8 e4m3,
     descaled in the z-eviction copy) so scores = h . (A h)_shifted.
     all weights pack into 2 arrays (fp8 matrices / f32 scalars).
     attention runs per 128-pixel chunk: Gram bands G = h^T z go to DRAM,
     then per group of 8 chunks strided gather DMAs extract all 9 score
     diagonals, one batched softmax normalizes them, and strided scatter
     DMAs build the banded W' matrices in DRAM; each chunk then does one
     banded-matmul weighted v-sum, h_new^T via identity matmul in the
     same PSUM tile, and evicts delta = psum - x straight to fp8.
     the device returns delta = out - h (small magnitude); the host
     reconstructs out = h_f32 + delta at full precision.
"""
import math
import time

import numpy as np
import ml_dtypes

import concourse.bass as bass
import concourse.tile as tile
from concourse import bacc, mybir
from concourse.bass_utils import run_bass_kernel_spmd

B, H, W, C = 16, 64, 64, 256
NCORES = 8
BS = B // NCORES          # images per core
C2, C3 = 2 * C, 3 * C
HW = H * W                # 4096 pixels per image
NT = 8                    # 512-pixel tiles per image
NCHUNK = HW // 128        # 32 x 128-pixel chunks per image
ZP = 1 + 66 * 64 + 1      # padded-z flat length (guard + 66 rows + guard)
W1SC = 16.0               # fp8 shipping scale for W1 (descaled in GELU)
W2SC = 16.0               # fp8 shipping scale for W2/Wv (descaled on evict)
ASC = 256.0               # fp8 shipping scale for A (descaled in z copy)
GB = 128 * 258            # Gram band size (f32 elements per chunk)
WB = 384 * 128            # banded W' size (bf16 elements per chunk)

F32 = mybir.dt.float32
BF16 = mybir.dt.bfloat16
F8E3 = mybir.dt.float8e3   # e3m4: max 15.5, best for ~N(0,1) data
F8E4 = mybir.dt.float8e4   # e4m3

NP_E3 = ml_dtypes.float8_e3m4
NP_E4 = ml_dtypes.float8_e4m3
NP_BF = ml_dtypes.bfloat16

_TAUS = [(dy, dx) for dy in (-1, 0, 1) for dx in (-1, 0, 1)]
DS = bass.ds


def _capr(ap, extra, dims):
    """Rebuild an AP's dims, keeping its (possibly symbolic) offset + extra."""
    a = ap.copy()
    if extra:
        a.offset = a.offset + extra
    v = a.ap
    v.clear()
    v.extend([(int(s), int(n)) for (s, n) in dims])
    return a


def _view(ap, dims, extra=None):
    """Reshape the FREE dims of an SBUF/PSUM AP (keeps the partition dim),
    optionally advancing the offset by `extra` (may be symbolic)."""
    a = ap.copy()
    if extra is not None:
        a.offset = a.offset + extra
    v = a.ap
    p0 = tuple(v[0])
    v.clear()
    v.extend([p0] + [(int(s), int(n)) for (s, n) in dims])
    return a


def _build_program(reps=1):
    nc = bacc.Bacc(
        trn_type="TRN2", target_bir_lowering=False, debug=False,
        num_devices=NCORES,
    )
    # ---- DRAM I/O (per-core), wire-compressed dtypes, 4 arrays.
    hraw_d = nc.dram_tensor("hraw", [BS, 128, 32, 256], F8E3,
                            kind="ExternalInput").ap()
    # wpk8: [w1k x16 (24x128) | at x256 (4x128) | i256 (2x256) |
    #        w2t x16 (8x128) | wvt x16 (2x256)] e4m3
    wpk8_d = nc.dram_tensor("wpk8", [128, 5632], F8E4,
                            kind="ExternalInput").ap()
    # bk: [bh (4) | b2 x16 (2) | mask (9) | taps (54)] f32
    bk_d = nc.dram_tensor("bk", [128, 69], F32, kind="ExternalInput").ap()
    out_d = nc.dram_tensor("out", [BS, 64, 64, 256], F8E3,
                           kind="ExternalOutput").ap()

    GELU = mybir.ActivationFunctionType.Gelu
    EXP = mybir.ActivationFunctionType.Exp
    COPY = mybir.ActivationFunctionType.Copy
    ADD = mybir.AluOpType.add
    MULT = mybir.AluOpType.mult
    SUB = mybir.AluOpType.subtract

    with tile.TileContext(nc) as tc:
        with (
            tc.tile_pool(name="wts", bufs=1) as wts,
            tc.tile_pool(name="konst", bufs=1) as konst,
            tc.tile_pool(name="csc", bufs=2) as csc,
            tc.tile_pool(name="percp", bufs=1) as percp,
            tc.tile_pool(name="stg", bufs=2) as stg,
            tc.tile_pool(name="data", bufs=2) as data,
            tc.tile_pool(name="hnewp", bufs=2) as hnewp,
            tc.tile_pool(name="zpadp", bufs=2) as zpadp,
            tc.tile_pool(name="hidp", bufs=8) as hidp,
            tc.tile_pool(name="vap", bufs=1) as vap,
            tc.tile_pool(name="small", bufs=4) as small,
            tc.tile_pool(name="wlp", bufs=6) as wlp,
            tc.tile_pool(name="ps1", bufs=2, space="PSUM") as ps1,
            tc.tile_pool(name="ps2", bufs=1, space="PSUM") as ps2,
            tc.tile_pool(name="ps3", bufs=2, space="PSUM") as ps3,
            tc.tile_pool(name="gdram", bufs=2, space="DRAM") as gdram,
            tc.tile_pool(name="wpdram", bufs=2, space="DRAM") as wpdram,
        ):
            # ---------- load packed weights / constants ----------
            wpk8 = wts.tile([128, 5632], F8E4, name="wpk8")
            nc.sync.dma_start(wpk8[:], wpk8_d[:])
            bk = konst.tile([128, 69], F32, name="bk")
            nc.sync.dma_start(bk[:], bk_d[:])

            def w1k_ap(kidx, mc):
                c0 = (kidx * 4 + mc) * 128
                return wpk8[:, c0:c0 + 128]

            def at_ap(kc, mc):
                c0 = 3072 + (kc * 2 + mc) * 128
                return wpk8[:, c0:c0 + 128]

            def i256_ap(kc):
                return wpk8[:, 3584 + kc * 256:3584 + kc * 256 + 256]

            i128_ap = wpk8[:, 3584:3584 + 128]

            def w2t_ap(kc, mc):
                c0 = 4096 + (kc * 2 + mc) * 128
                return wpk8[:, c0:c0 + 128]

            def wvt_ap(kc):
                return wpk8[:, 5120 + kc * 256:5120 + kc * 256 + 256]

            def bh_ap(mc):
                return bk[:, mc:mc + 1]

            def b2_ap(mc):
                return bk[:, 4 + mc:5 + mc]

            mask_ap = bk[:, 6:15]

            vzero = konst.tile([128, 256], BF16, name="vzero")
            nc.gpsimd.memset(vzero[:], 0.0)
            wpz = konst.tile([128, 3072], BF16, name="wpz")
            nc.gpsimd.memset(wpz[:], 0.0)
            maskrep = konst.tile([128, 72], F32, name="maskrep")
            for g in range(8):
                nc.vector.tensor_copy(maskrep[:, 9 * g:9 * g + 9], mask_ap)

            # ---------- per-image pipeline (body of the image For_i) -----
            def run_image(img):
                # NHWC chunks [pix128, chunk32, ch256], contiguous DMA
                hh = data.tile([128, 32, 256], F8E3, name="hh", tag="hh")
                nc.sync.dma_start(hh[:], hraw_d[DS(img, 1)])
                # transpose+pad to channel-major [128ch, 66, 66]
                xr = []
                for cc in range(2):
                    t = data.tile([128, 66, 66], F8E3, name="xr", tag="xr")
                    nc.gpsimd.memset(t[:], 0.0)
                    xr.append(t)
                with tc.For_i(0, 32) as p:
                    hhs = stg.tile([128, 256], F8E3, name="hhs", tag="hhs")
                    nc.vector.tensor_copy(hhs[:], hh[:, DS(p, 1), :])
                    tp = ps2.tile([128, 256], F32, space="PSUM",
                                  name="fin_ps", tag="fin_ps")
                    for cc in range(2):
                        nc.tensor.matmul(
                            tp[:, 128 * cc:128 * cc + 128],
                            hhs[:, 128 * cc:128 * cc + 128],
                            i128_ap, start=True, stop=True)
                    for cc in range(2):
                        nc.vector.tensor_copy(
                            xr[cc][:, DS(p * 2 + 1, 2), 1:65],
                            tp[:, 128 * cc:128 * cc + 128])

                h_new = []
                for cc in range(2):
                    h_new.append(hnewp.tile([128, HW], BF16, name="h_new",
                                            tag="h_new"))
                z_pad = []
                for cc in range(2):
                    zt = zpadp.tile([128, ZP], BF16, name="z_pad",
                                    tag="z_pad")
                    nc.gpsimd.memset(zt[:, 0:65], 0.0)
                    nc.gpsimd.memset(zt[:, ZP - 65:ZP], 0.0)
                    z_pad.append(zt)

                # ---- ST1: DVE depthwise conv -> up1 -> GELU -> up2 ----
                perc = percp.tile([128, 12 * 2048], BF16, name="perc")
                with tc.For_i(0, 3) as t:
                    for cc in range(2):
                        for half in range(2):
                            prev = None
                            for tau, (dy, dx) in enumerate(_TAUS):
                                in0 = xr[cc][:, 1 + dy + 32 * half:
                                             33 + dy + 32 * half,
                                             1 + dx:65 + dx]
                                tapc = bk[:, DS(15 + cc * 27 + t * 9 + tau,
                                                1)]
                                if tau == 0:
                                    cur = csc.tile([128, 2048], F32,
                                                   name="cacc", tag="cacc")
                                    nc.vector.tensor_scalar(
                                        out=cur[:], in0=in0, scalar1=tapc,
                                        scalar2=None, op0=MULT)
                                elif tau < 8:
                                    cur = csc.tile([128, 2048], F32,
                                                   name="cacc", tag="cacc")
                                    nc.vector.scalar_tensor_tensor(
                                        out=cur[:], in0=in0, scalar=tapc,
                                        in1=prev[:], op0=MULT, op1=ADD)
                                else:
                                    nc.vector.scalar_tensor_tensor(
                                        out=perc[:, DS((t + 3 * cc) * 4096 +
                                                       half * 2048, 2048)],
                                        in0=in0, scalar=tapc, in1=prev[:],
                                        op0=MULT, op1=ADD)
                                prev = cur
                with tc.For_i(0, NT) as nt:
                    hid_sb = []
                    for mc in range(4):
                        hp = ps1.tile([128, 512], F32, space="PSUM",
                                      name="hid_ps", tag="hid_ps")
                        for kidx in range(6):
                            nc.tensor.matmul(
                                hp[:], w1k_ap(kidx, mc),
                                perc[:, DS(kidx * 4096 + nt * 512, 512)],
                                start=(kidx == 0), stop=(kidx == 5))
                        hs = hidp.tile([128, 512], BF16, name="hid_sb",
                                       tag="hid_sb")
                        nc.scalar.activation(hs[:], hp[:], GELU,
                                             bias=bh_ap(mc),
                                             scale=1.0 / W1SC)
                        hid_sb.append(hs)
                    for mc in range(2):
                        dp = ps2.tile([128, 512], F32, space="PSUM",
                                      name="dx_ps", tag="dx_ps")
                        for kc in range(4):
                            nc.tensor.matmul(
                                dp[:], w2t_ap(kc, mc), hid_sb[kc][:],
                                start=(kc == 0), stop=(kc == 3))
                        xres = _view(
                            xr[mc][:], [(66, 8), (1, 64)],
                            extra=(nt * 8 + 1) * 66 + 1)
                        dsc = csc.tile([128, 512], F32, name="dsc",
                                       tag="dsc")
                        nc.vector.tensor_scalar(
                            out=dsc[:], in0=dp[:], scalar1=b2_ap(mc),
                            scalar2=1.0 / W2SC, op0=ADD, op1=MULT)
                        nc.vector.tensor_tensor(
                            h_new[mc][:, DS(nt * 512, 512)],
                            dsc[:], xres, op=ADD)
                    for mc in range(2):
                        zps = ps2.tile([128, 512], F32, space="PSUM",
                                       name="z_ps", tag="z_ps")
                        for kc in range(2):
                            nc.tensor.matmul(
                                zps[:], at_ap(kc, mc),
                                h_new[kc][:, DS(nt * 512, 512)],
                                start=(kc == 0), stop=(kc == 1))
                        nc.scalar.activation(
                            z_pad[mc][:, DS(nt * 512 + 65, 512)],
                            zps[:], COPY, scale=1.0 / ASC)

                # ---- attention part 1: v tiles + Gram bands to DRAM ----
                # v_all has one zeroed 256-col guard block on each side
                v_all = vap.tile([128, 34 * 256], BF16, name="v_all",
                                 tag="v_all")
                nc.gpsimd.memset(v_all[:, 0:256], 0.0)
                nc.gpsimd.memset(v_all[:, 33 * 256:34 * 256], 0.0)
                gall = gdram.tile([NCHUNK, 128, 258], F32, space="DRAM",
                                  name="gall", tag="gall")
                with tc.For_i(0, NCHUNK) as k:
                    hst = []
                    for cc in range(2):
                        hs = stg.tile([128, 128], BF16, name="hst",
                                      tag=f"hst{cc}")
                        nc.vector.tensor_copy(
                            hs[:], h_new[cc][:, DS(k * 128, 128)])
                        hst.append(hs)
                    vps = ps2.tile([128, 256], F32, space="PSUM",
                                   name="v_ps", tag="v_ps")
                    for kc in range(2):
                        nc.tensor.matmul(vps[:], hst[kc][:], wvt_ap(kc),
                                         start=(kc == 0), stop=(kc == 1))
                    nc.scalar.activation(
                        v_all[:, DS(k * 256 + 256, 256)], vps[:], COPY,
                        scale=1.0 / W2SC)
                    gps = ps3.tile([128, 258], F32, space="PSUM",
                                   name="g_ps", tag="g_ps")
                    for kc in range(2):
                        nc.tensor.matmul(
                            gps[:], hst[kc][:],
                            z_pad[kc][:, DS(k * 128, 258)],
                            start=(kc == 0), stop=(kc == 1))
                    gsb = small.tile([128, 258], F32, name="gsb", tag="gsb")
                    nc.scalar.activation(gsb[:], gps[:], COPY)
                    nc.sync.dma_start(gall[DS(k, 1)], gsb[:])

                # ---- attention part 2: batched softmax + W' (8/group) ----
                wall = wpdram.tile([NCHUNK, 384, 128], BF16, space="DRAM",
                                   name="wall", tag="wall")
                with tc.For_i(0, 4) as g:
                    nc.sync.dma_start(
                        _capr(wall[DS(g * 8, 8)], 0,
                              [(1, 128), (WB, 8), (128, 384)]),
                        wpz[:])
                    sca = small.tile([128, 72], F32, name="sca", tag="sca")
                    gb = gall[DS(g * 8, 8)]
                    for a in range(3):
                        nc.sync.dma_start(
                            _view(sca[:, 3 * a:], [(9, 8), (1, 3)]),
                            _capr(gb, 64 * a,
                                  [(259, 128), (GB, 8), (1, 3)]))
                    sm = small.tile([128, 72], F32, name="sm", tag="sm")
                    nc.vector.tensor_tensor(sm[:], sca[:], maskrep[:],
                                            op=MULT)
                    ex = small.tile([128, 72], F32, name="ex", tag="ex")
                    nc.scalar.activation(ex[:], sm[:], EXP)
                    sume = small.tile([128, 8], F32, name="sume", tag="sume")
                    nc.vector.tensor_reduce(
                        sume[:], _view(ex[:], [(9, 8), (1, 9)]),
                        axis=mybir.AxisListType.X, op=ADD)
                    rec = small.tile([128, 8], F32, name="rec", tag="rec")
                    nc.vector.reciprocal(rec[:], sume[:])
                    exn = small.tile([128, 72], F32, name="exn", tag="exn")
                    nc.vector.tensor_tensor(
                        exn[:], ex[:], _view(rec[:], [(1, 8), (0, 9)]),
                        op=MULT)
                    wn = small.tile([128, 72], BF16, name="wn", tag="wn")
                    nc.vector.tensor_tensor(wn[:], exn[:], maskrep[:],
                                            op=MULT)
                    wb = wall[DS(g * 8, 8)]
                    for a in range(3):
                        for b in range(3):
                            nc.sync.dma_start(
                                _capr(wb, 8064 + 8192 * a + 128 * b,
                                      [(129, 128), (WB, 8)]),
                                _view(wn[:, 3 * a + b:], [(9, 8)]))

                # ---- attention part 3: weighted v-sum + h^T -> delta ----
                with tc.For_i(0, NCHUNK) as j:
                    wl3 = wlp.tile([128, 384], BF16, name="wl3", tag="wl3")
                    nc.sync.dma_start(
                        wl3[:],
                        _capr(wall[DS(j, 1)], 0,
                              [(128, 128), (16384, 3), (1, 128)]))
                    hst = []
                    for cc in range(2):
                        hs = stg.tile([128, 128], BF16, name="hst3",
                                      tag=f"hst3{cc}")
                        nc.vector.tensor_copy(
                            hs[:], h_new[cc][:, DS(j * 128, 128)])
                        hst.append(hs)
                    fp = ps2.tile([128, 256], F32, space="PSUM",
                                  name="fin_ps", tag="fin_ps")
                    for kc in range(2):
                        nc.tensor.matmul(fp[:], hst[kc][:], i256_ap(kc),
                                         start=(kc == 0), stop=False)
                    for j3 in range(3):
                        nc.tensor.matmul(
                            fp[:], wl3[:, 128 * j3:128 * j3 + 128],
                            v_all[:, DS(j * 256 + j3 * 256, 256)],
                            start=False, stop=(j3 == 2))
                    osb = small.tile([128, 256], F8E3, name="osb", tag="osb")
                    nc.vector.tensor_tensor(osb[:], fp[:],
                                            hh[:, DS(j, 1), :], op=SUB)
                    nc.sync.dma_start(
                        out_d[DS(img, 1), DS(j * 2, 2)], osb[:])

            if reps == 1:
                with tc.For_i(0, BS) as img:
                    run_image(img)
            else:
                for img in [i % BS for i in range(BS * reps)]:
                    run_image(img)

    nc.compile()
    return nc


_NC_CACHE = {}


def _get_program():
    if "nc" not in _NC_CACHE:
        _NC_CACHE["nc"] = _build_program()
    return _NC_CACHE["nc"]


def _host_prepare(w_perc, b_perc, w_up1, b_up1, w_up2, b_up2, w_qkv, b_qkv):
    w_perc = np.asarray(w_perc, np.float32)
    b_perc = np.asarray(b_perc, np.float32)
    w_up1 = np.asarray(w_up1, np.float32)
    b_up1 = np.asarray(b_up1, np.float32)
    w_up2 = np.asarray(w_up2, np.float32)
    b_up2 = np.asarray(b_up2, np.float32)
    w_qkv = np.asarray(w_qkv, np.float32)
    b_qkv = np.asarray(b_qkv, np.float32)
    assert np.allclose(b_qkv, 0.0), "kernel assumes zero qkv bias (A-trick)"

    wp = w_perc[:, 0]                       # [3C, 3, 3]
    W1 = w_up1[:, :, 0, 0]                  # [2C, 3C]
    bh = b_up1 + W1 @ b_perc                # [2C]
    W2 = w_up2[:, :, 0, 0]                  # [C, 2C]
    Wq, Wk, Wv = w_qkv[:C], w_qkv[C:C2], w_qkv[C2:]
    A = (Wq.T @ Wk) / math.sqrt(C)          # [C, C]

    # wpk8 = [w1k x16 | at x ASC | i256 | w2t x16 | wvt x16] e4m3
    # perceived k-chunk kidx = cc*3+t holds channels g of group (128cc+g, t)
    W1r = W1.reshape(C2, C, 3)              # [d, g, t]
    wpk8 = np.empty((128, 5632), np.float32)
    for cc in range(2):
        for t in range(3):
            kidx = cc * 3 + t
            for mc in range(4):
                c0 = (kidx * 4 + mc) * 128
                wpk8[:, c0:c0 + 128] = (
                    W1SC * W1r[mc * 128:(mc + 1) * 128,
                               cc * 128:(cc + 1) * 128, t].T)
    for kc in range(2):
        for mc in range(2):
            c0 = 3072 + (kc * 2 + mc) * 128
            wpk8[:, c0:c0 + 128] = \
                ASC * A[mc * 128:(mc + 1) * 128, kc * 128:(kc + 1) * 128].T
    eye = np.eye(256, dtype=np.float32).reshape(2, 128, 256)
    for kc in range(2):
        wpk8[:, 3584 + kc * 256:3584 + kc * 256 + 256] = eye[kc]
    for kc in range(4):
        for mc in range(2):
            c0 = 4096 + (kc * 2 + mc) * 128
            wpk8[:, c0:c0 + 128] = W2SC * \
                W2[mc * 128:(mc + 1) * 128, kc * 128:(kc + 1) * 128].T
    WvT = Wv.T.reshape(2, 128, 256)
    for kc in range(2):
        wpk8[:, 5120 + kc * 256:5120 + kc * 256 + 256] = W2SC * WvT[kc]

    # bk = [bh | b2 x16 | mask | taps]
    bk = np.zeros((128, 69), np.float32)
    bk[:, 0:4] = bh.reshape(4, 128).T
    bk[:, 4:6] = W2SC * b_up2.reshape(2, 128).T
    maskt = np.ones((128, 9), np.float32)
    for p in range(128):
        xx = p % 64
        for dy in (-1, 0, 1):
            for dx in (-1, 0, 1):
                if (xx == 0 and dx == -1) or (xx == 63 and dx == 1):
                    maskt[p, (dy + 1) * 3 + (dx + 1)] = 0.0
    bk[:, 6:15] = maskt
    wpr = wp.reshape(C, 3, 9)               # [g', t, tau]
    for cc in range(2):
        bk[:, 15 + cc * 27:15 + cc * 27 + 27] = \
            wpr[cc * 128:(cc + 1) * 128].reshape(128, 27)

    return dict(wpk8=wpk8.astype(NP_E4), bk=bk)


def _pack_h(h):
    """h [B,H,W,C] f32 -> per-core fp8 [BS,128,32,256] pixel-major chunks."""
    h8 = np.ascontiguousarray(h).astype(NP_E3)
    return [np.ascontiguousarray(
        h8[core * BS:(core + 1) * BS].reshape(BS, 32, 128, 256)
        .transpose(0, 2, 1, 3)) for core in range(NCORES)]


def kernel(h, w_perc, b_perc, w_up1, b_up1, w_up2, b_up2, w_qkv, b_qkv):
    h = np.asarray(h, np.float32)
    consts = _host_prepare(w_perc, b_perc, w_up1, b_up1, w_up2, b_up2,
                           w_qkv, b_qkv)
    nc = _get_program()

    in_maps = []
    for hraw in _pack_h(h):
        m = {"hraw": hraw}
        m.update(consts)
        in_maps.append(m)

    res = None
    for attempt in range(3):
        try:
            res = run_bass_kernel_spmd(nc, in_maps,
                                       core_ids=list(range(NCORES)),
                                       trace=False)
            break
        except Exception:
            if attempt == 2:
                raise
            time.sleep(2.0)
    delta = np.concatenate([res.results[i]["out"] for i in range(NCORES)], 0)
    return h + delta.astype(np.float32)
